# revision 1
# baseline (speedup 1.0000x reference)
"""Trainium2 Bass kernel for ConvexDisplacementUpdate (B=4, L=4096, D=256).

new_coords = alpha * softmax(10 * qhat @ khat^T) @ coords + (1-alpha) * coords
q = l2norm(latents @ Wq^T), k = l2norm(latents @ Wk^T)  (row-wise l2norm)

Strategy (flash-attention style; the [L, L] score matrix never touches HBM):
  - 8 cores = (4 batches) x (2 query halves of 2048 rows). Host rolls each
    core's per-batch data so its own query rows are always columns 0:2048
    of the transposed latents -> one SPMD program, no per-core control flow.
  - Scores are computed transposed, S^T[m, l] = k_m . qhat_l, with k left
    UN-normalized; the per-m factor 10/||k_m|| is a per-partition scale
    folded into the exp() activation.
  - softmax without max-subtraction (|scores| <= 10, exp is safe in fp32).
  - numerator and denominator come from one PE matmul per tile with the
    ones-augmented coords [x, y, 1] as the stationary operand, accumulated
    over all 32 m-tiles in PSUM.
  - final alpha-blend + division happen on host (B*L*2 elements, trivial).
"""

import numpy as np

B, L, D = 4, 4096, 256
HALF = L // 2  # 2048 query rows per core
NCORES = 8
INV_TEMP = 10.0

_CACHE = {}


def build_module(reps=1, use_f32r=True, phases=3, loop_n=0, qk_bf16=True):
    """Build + compile the SPMD Bass module (one program, 8 cores)."""
    from contextlib import ExitStack

    import concourse.bacc as bacc
    import concourse.mybir as mybir
    import concourse.tile as tile
    from concourse.bass import ts
    from concourse.masks import make_identity

    dt = mybir.dt
    f32 = dt.float32
    AF = mybir.ActivationFunctionType
    ALU = mybir.AluOpType

    fr = dt.float32r if use_f32r else f32
    qk = dt.bfloat16 if qk_bf16 else fr

    def mm(ap):
        return ap

    nc = bacc.Bacc("TRN2", target_bir_lowering=False, debug=False,
                   num_devices=NCORES)

    latT = nc.dram_tensor("latT", [D, L], f32, kind="ExternalInput")
    wqT_d = nc.dram_tensor("wqT", [D, D], f32, kind="ExternalInput")
    wkT_d = nc.dram_tensor("wkT", [D, D], f32, kind="ExternalInput")
    caug_hi_d = nc.dram_tensor("caug_hi", [128, 3 * (L // 128)], dt.bfloat16,
                               kind="ExternalInput")
    caug_lo_d = nc.dram_tensor("caug_lo", [128, 3 * (L // 128)], dt.bfloat16,
                               kind="ExternalInput")
    pv_d = nc.dram_tensor("pv", [3, HALF], f32, kind="ExternalOutput")

    NLT = L // 128        # 32 m-tiles
    NQT = HALF // 128     # 16 q l-tiles
    NMB = L // 512        # 8 m-blocks
    NLB = HALF // 512     # 4 l-blocks

    with tile.TileContext(nc) as tc:
        loop = tc.For_i(0, loop_n, 1) if loop_n else None
        if loop is not None:
            loop.__enter__()
        for _rep in range(reps):
            with ExitStack() as ctx:
                persist = ctx.enter_context(tc.tile_pool(name="persist", bufs=1))

                # ---- load inputs (small weights first, lat chunks
                # interleaved across the two d-tiles so the first matmuls
                # can start after ~1MB) ----
                wq = [persist.tile([128, D], fr, tag=f"wq{i}", name=f"wq{i}") for i in range(2)]
                wk = [persist.tile([128, D], fr, tag=f"wk{i}", name=f"wk{i}") for i in range(2)]
                for i in range(2):
                    nc.sync.dma_start(out=wq[i], in_=wqT_d[i * 128:(i + 1) * 128, :].bitcast(fr))
                    nc.sync.dma_start(out=wk[i], in_=wkT_d[i * 128:(i + 1) * 128, :].bitcast(fr))
                caug_hi = persist.tile([128, 3 * NLT], dt.bfloat16, tag="caug_hi")
                caug_lo = persist.tile([128, 3 * NLT], dt.bfloat16, tag="caug_lo")
                nc.sync.dma_start(out=caug_hi, in_=caug_hi_d[:, :])
                nc.sync.dma_start(out=caug_lo, in_=caug_lo_d[:, :])
                ident = persist.tile([128, 128], f32, tag="ident")
                make_identity(nc, ident)
                ones = persist.tile([128, 1], f32, tag="ones")
                nc.vector.memset(ones, 1.0)

                lat = [persist.tile([128, L], fr, tag=f"lat{i}", name=f"lat{i}") for i in range(2)]
                chunks = [(0, 512), (512, 512), (1024, 1024), (2048, 1024),
                          (3072, 1024)]
                for off, size in chunks:
                    for i in range(2):
                        nc.sync.dma_start(
                            out=lat[i][:, off:off + size],
                            in_=latT[i * 128:(i + 1) * 128,
                                     off:off + size].bitcast(fr))

                qT = [persist.tile([128, HALF], qk, tag=f"qT{i}", name=f"qT{i}") for i in range(2)]
                kT = [persist.tile([128, L], qk, tag=f"kT{i}", name=f"kT{i}") for i in range(2)]
                q_all = persist.tile([128, NQT * D], f32, tag="q_all")
                ssq_q = persist.tile([128, NQT], f32, tag="ssq_q")
                inv_q = persist.tile([128, NQT], f32, tag="inv_q")
                inv_kT = persist.tile([128, NLT], f32, tag="inv_kT")

                with ExitStack() as p1:
                    big_ps = p1.enter_context(
                        tc.tile_pool(name="big_ps", bufs=3, space="PSUM"))
                    tp_ps = p1.enter_context(
                        tc.tile_pool(name="tp_ps", bufs=2, space="PSUM"))
                    kssq_ps = p1.enter_context(
                        tc.tile_pool(name="kssq_ps", bufs=1, space="PSUM"))
                    sm = p1.enter_context(tc.tile_pool(name="p1_small", bufs=4))
                    qh_pool = p1.enter_context(tc.tile_pool(name="qhat", bufs=3))
                    sq_pool = p1.enter_context(tc.tile_pool(name="k_sq", bufs=4))

                    # ---- phase 1q-A: raw q [l, e] + row sum-squares
                    # (ACT Square + accum_out straight from PSUM) ----
                    for lt in range(NQT):
                        qle = big_ps.tile([128, D], f32, tag="big", name="qle")
                        nc.tensor.matmul(qle, mm(lat[0][:, ts(lt, 128)]),
                                         mm(wq[0]), start=True, stop=False)
                        nc.tensor.matmul(qle, mm(lat[1][:, ts(lt, 128)]),
                                         mm(wq[1]), start=False, stop=True)
                        nc.vector.tensor_copy(out=q_all[:, ts(lt, D)], in_=qle)
                        junk = sm.tile([128, D], f32, tag="sqj")
                        nc.scalar.activation(junk, qle, AF.Square,
                                             accum_out=ssq_q[:, lt:lt + 1])
                    nrm_q = persist.tile([128, NQT], f32, tag="nrm_q")
                    nc.scalar.activation(nrm_q, ssq_q, AF.Sqrt)
                    nc.vector.reciprocal(inv_q, nrm_q)

                    # ---- phase 1q-B: normalize + transpose to [e, l] ----
                    for lt in range(NQT):
                        qhat = qh_pool.tile([128, D], f32, tag="qhat")
                        nc.vector.tensor_scalar_mul(qhat, q_all[:, ts(lt, D)],
                                                    inv_q[:, lt:lt + 1])
                        for et in range(2):
                            tp = tp_ps.tile([128, 128], f32, tag="tp")
                            nc.tensor.transpose(tp, qhat[:, ts(et, 128)], ident)
                            nc.vector.tensor_copy(out=qT[et][:, ts(lt, 128)], in_=tp)

                    # ---- phase 1k: kT_raw [e, m]; ssq via N=1 matmuls
                    # straight into the transposed [m-tile] layout ----
                    kssq = kssq_ps.tile([128, NLT], f32, tag="kssq")
                    for mb in range(NMB):
                        sqs = []
                        for et in range(2):
                            kp = big_ps.tile([128, 512], f32, tag="big", name="kp")
                            nc.tensor.matmul(kp, mm(wk[0][:, ts(et, 128)]),
                                             mm(lat[0][:, ts(mb, 512)]),
                                             start=True, stop=False)
                            nc.tensor.matmul(kp, mm(wk[1][:, ts(et, 128)]),
                                             mm(lat[1][:, ts(mb, 512)]),
                                             start=False, stop=True)
                            nc.vector.tensor_copy(out=kT[et][:, ts(mb, 512)],
                                                  in_=kp)
                            sq = sq_pool.tile([128, 512], f32, tag="ksq")
                            nc.vector.tensor_mul(sq, kT[et][:, ts(mb, 512)],
                                                 kT[et][:, ts(mb, 512)])
                            sqs.append(sq)
                        for j in range(4):
                            col = 4 * mb + j
                            for et in range(2):
                                nc.tensor.matmul(kssq[:, col:col + 1],
                                                 sqs[et][:, ts(j, 128)], ones,
                                                 start=(et == 0), stop=(et == 1))
                    # 10/||k_m||: 1/sqrt(ssq/100)
                    nrm_k = persist.tile([128, NLT], f32, tag="nrm_k")
                    nc.scalar.activation(nrm_k, kssq, AF.Sqrt,
                                         scale=1.0 / (INV_TEMP * INV_TEMP))
                    nc.vector.reciprocal(inv_kT, nrm_k)

                if phases < 3:
                    with tc.tile_pool(name="dbg", bufs=1) as dbg:
                        dtile = dbg.tile([3, HALF], f32, name="dtile")
                        nc.vector.tensor_copy(out=dtile, in_=kT[0][0:3, 0:HALF])
                        nc.sync.dma_start(out=pv_d[:, :], in_=dtile)
                    continue

                # ---- phase 2: scores^T -> exp -> [coords|1]^T @ P^T ----
                # software-pipelined: pv matmuls of tile t-1 are emitted
                # after the scores matmuls of tile t so PE never waits on
                # ACT's exp.
                with ExitStack() as p2:
                    sp_ps = p2.enter_context(
                        tc.tile_pool(name="sp_ps", bufs=3, space="PSUM"))
                    pv_ps = p2.enter_context(
                        tc.tile_pool(name="pv_ps", bufs=1, space="PSUM"))
                    p_pool = p2.enter_context(tc.tile_pool(name="p_sb", bufs=5))
                    pv_all = pv_ps.tile([128, 512], f32, tag="pv")

                    def emit_pv(t, ptiles):
                        for lb in range(NLB):
                            prhs = ptiles[lb // 2][:, ts(lb % 2, 512)]
                            nc.tensor.matmul(
                                pv_all[32 * lb:32 * lb + 3, :],
                                caug_hi[:, ts(t, 3)], prhs,
                                start=(t == 0), stop=False,
                                tile_position=(0, 32 * lb))
                            nc.tensor.matmul(
                                pv_all[32 * lb:32 * lb + 3, :],
                                caug_lo[:, ts(t, 3)], prhs,
                                start=False, stop=(t == NLT - 1),
                                tile_position=(0, 32 * lb))

                    prev = None
                    for t in range(NLT):
                        cur = []
                        for j in range(2):
                            sp = sp_ps.tile([128, 1024], f32, tag="sp")
                            for h in range(2):
                                lb = 2 * j + h
                                nc.tensor.matmul(sp[:, ts(h, 512)],
                                                 mm(kT[0][:, ts(t, 128)]),
                                                 mm(qT[0][:, ts(lb, 512)]),
                                                 start=True, stop=False)
                                nc.tensor.matmul(sp[:, ts(h, 512)],
                                                 mm(kT[1][:, ts(t, 128)]),
                                                 mm(qT[1][:, ts(lb, 512)]),
                                                 start=False, stop=True)
                            p = p_pool.tile([128, 1024], dt.bfloat16, tag="p")
                            nc.scalar.activation(p, sp, AF.Exp,
                                                 scale=inv_kT[:, t:t + 1])
                            cur.append(p)
                        if prev is not None:
                            emit_pv(t - 1, prev)
                        prev = cur
                    emit_pv(NLT - 1, prev)
                    out_sb = p2.enter_context(tc.tile_pool(name="out_sb", bufs=2))
                    for lb in range(NLB):
                        ot = out_sb.tile([3, 512], f32, tag="ot")
                        nc.vector.tensor_copy(out=ot,
                                              in_=pv_all[32 * lb:32 * lb + 3, :])
                        nc.sync.dma_start(out=pv_d[:, ts(lb, 512)], in_=ot)

        if loop is not None:
            loop.__exit__(None, None, None)
    nc.compile()
    return nc


def _get_module():
    if "nc" not in _CACHE:
        _CACHE["nc"] = build_module()
    return _CACHE["nc"]


def make_in_maps(latents, current_coords, Wq, Wk):
    """Per-core input dicts. Core c -> batch c//2, query half c%2 (rolled
    so own query rows are always columns 0:2048)."""
    latents = np.asarray(latents, np.float32)
    coords = np.asarray(current_coords, np.float32)
    wqT = np.ascontiguousarray(np.asarray(Wq, np.float32).T)
    wkT = np.ascontiguousarray(np.asarray(Wk, np.float32).T)
    in_maps = []
    for c in range(NCORES):
        b, h = divmod(c, 2)
        lat_b = np.roll(latents[b], -HALF * h, axis=0)
        coo_b = np.roll(coords[b], -HALF * h, axis=0)
        aug = np.concatenate([coo_b, np.ones((L, 1), np.float32)], axis=1)
        caug = np.ascontiguousarray(
            aug.reshape(L // 128, 128, 3).transpose(1, 0, 2).reshape(128, -1))
        import ml_dtypes
        hi = caug.astype(ml_dtypes.bfloat16)
        lo = (caug - hi.astype(np.float32)).astype(ml_dtypes.bfloat16)
        in_maps.append({
            "latT": np.ascontiguousarray(lat_b.T),
            "wqT": wqT,
            "wkT": wkT,
            "caug_hi": hi,
            "caug_lo": lo,
        })
    return in_maps


def postprocess(results, current_coords, alpha):
    """Assemble (new_coords, displacement) from per-core pv = [num_x; num_y; den]."""
    coords = np.asarray(current_coords, np.float32)
    new_coords = np.empty((B, L, 2), np.float32)
    for c in range(NCORES):
        b, h = divmod(c, 2)
        pv = results[c]["pv"]
        wc = (pv[0:2, :] / pv[2:3, :]).T  # [2048, 2] = (W @ coords) rows
        rows = slice(h * HALF, (h + 1) * HALF)
        new_coords[b, rows] = alpha * wc + (1.0 - alpha) * coords[b, rows]
    displacement = new_coords - coords
    return new_coords, displacement


def kernel(latents, current_coords, Wq, Wk, alpha_raw, layer_idx=None):
    from concourse.bass_utils import run_bass_kernel_spmd

    nc = _get_module()
    in_maps = make_in_maps(latents, current_coords, Wq, Wk)
    res = run_bass_kernel_spmd(nc, in_maps, list(range(NCORES)))
    alpha = np.float32(1.0 / (1.0 + np.exp(-np.float64(np.asarray(alpha_raw)))))
    return postprocess(res.results, current_coords, alpha)

